# revision 27
# baseline (speedup 1.0000x reference)
"""GPT forward pass on 8 TRN2 NeuronCores.

Sharding: core c -> batch b = c // 2, rank r = c % 2 owns tokens t with
t % 2 == r (even/odd interleave of the sequence).  The residual stream is
core-local in D-major layout (h^T: [D partition-chunks, 512 own tokens]).

Attention uses rank-pure key blocks: key block (src, m) holds one rank's
local keys 128m..128m+127 (own rank: src=0 from local kloc/vloc; peer
rank: src=1 from the gathered KTp/VA2p).  Causality at 128-block
granularity is uniform across cores: query block j needs key blocks
m <= j of both ranks, with the diagonal m == j masked by a data-driven
[2,128,128] 0/1 mask (own: k<=i; peer: r=0 -> k<i, r=1 -> k<=i).  This
cuts S/PV work from 2*32 to 2*20 matmul units per head per layer.
S matmuls process a HEAD PAIR per instruction (both heads of a d-chunk
share the K stationary): moving QT2[:, 2t:2t+2, suffix], suffix widths
512,384,256,128.  PV accumulates the same suffixes into one PSUM bank
per head.

Per layer two AllGathers over the core pair exchange K^T and the
PV-packed V ([V_h | ones] blocks); only the PEER half is landed, via a
partition_id()-derived dynamic DRAM offset.  Q runs between the gather
issue and the first peer-dependent matmul.

Weights are pre-packed on the host so every weight DMA is
partition-major contiguous (2-8KB per partition per transfer).
"""

import sys

sys.path.insert(0, "/opt/trn_rl_repo")

import dataclasses
import numpy as np
import ml_dtypes

import concourse.bass as bass
import concourse.bacc as bacc
import concourse.mybir as mybir
from concourse import tile
from concourse.bass_utils import run_bass_kernel_spmd

B, T, E, D, NH, DH, NL, FF, AD = 4, 1024, 512, 1024, 16, 64, 8, 4096, 8
TH = T // 2          # tokens per core
NC = 8
DCH = D // 128       # 8 partition chunks of the embedding dim
EPS = 1e-5
BF = mybir.dt.bfloat16
F32 = mybir.dt.float32
F32R = mybir.dt.float32r
AluOp = mybir.AluOpType
Act = mybir.ActivationFunctionType

# V packing: per rank, per head: 4 key blocks of [V_h(64) | ones(1)];
# the 128-wide PV stationary slice reads up to 63 junk cols past block 3.
VHB = 4 * 65                 # 260 cols per head
VW = NH * VHB + 200          # 4360 cols per rank (pad for copy/read APs)
KW = DCH * TH                # 4096 cols of K^T per rank

_cache = {}


def _build_program():
    nc = bacc.Bacc("TRN2", target_bir_lowering=False, debug=False, num_devices=NC)

    # --- DRAM parameters (identical graph on all cores; data differs) ---
    p_lcdT = nc.declare_dram_parameter("lcdT", [E, TH], BF, isOutput=False)
    p_actT = nc.declare_dram_parameter("actT", [AD, TH], F32, isOutput=False)
    p_posT = nc.declare_dram_parameter("posT", [D, TH], F32, isOutput=False)
    p_we = nc.declare_dram_parameter("W_embed", [4, 128, 4, 128], BF, isOutput=False)
    p_wa = nc.declare_dram_parameter("W_act", [AD, D // 2], F32, isOutput=False)
    p_wq = nc.declare_dram_parameter("Wq", [NL, 8, 128, 8, 128], BF, isOutput=False)
    p_wk = nc.declare_dram_parameter("Wk", [NL, 8, 128, 8, 128], BF, isOutput=False)
    p_wv = nc.declare_dram_parameter("Wv", [NL, 2, 128, 8, 512], BF, isOutput=False)
    p_wp = nc.declare_dram_parameter("Wp", [NL, 8, 128, 8, 128], BF, isOutput=False)
    p_w1 = nc.declare_dram_parameter("W1", [NL, 16, 128, 8, 256], BF, isOutput=False)
    p_w2 = nc.declare_dram_parameter("W2", [NL, 8, 128, 32, 128], BF, isOutput=False)
    p_wh = nc.declare_dram_parameter("Wh", [128, 8, E], BF, isOutput=False)
    p_mask = nc.declare_dram_parameter("mask4", [2, 2, 128, 128], BF, isOutput=False)
    p_ones = nc.declare_dram_parameter("ones128", [128, 128], F32R, isOutput=False)
    p_out = nc.declare_dram_parameter("out", [TH, E], F32, isOutput=True)

    with tile.TileContext(nc) as tc:
        # ---------------- pools ----------------
        const = tc.alloc_tile_pool(name="const", bufs=1)
        persist = tc.alloc_tile_pool(name="persist", bufs=1)
        zpool = tc.alloc_tile_pool(name="zpool", bufs=1)
        big = tc.alloc_tile_pool(name="bigact", bufs=1)
        wpool = tc.alloc_tile_pool(name="wpool", bufs=6)
        w1pool = tc.alloc_tile_pool(name="w1pool", bufs=3)
        w8pool = tc.alloc_tile_pool(name="w8pool", bufs=3)
        whpool = tc.alloc_tile_pool(name="whpool", bufs=1)
        tmp = tc.alloc_tile_pool(name="tmp", bufs=3)
        stat = tc.alloc_tile_pool(name="stat", bufs=3)
        ptp = tc.alloc_tile_pool(name="ptp", bufs=6)
        dram = tc.alloc_tile_pool(name="dram", bufs=2, space="DRAM")
        pp_mm = tc.alloc_tile_pool(name="pp_mm", bufs=2, space="PSUM")
        pp_pair = tc.alloc_tile_pool(name="pp_pair", bufs=2, space="PSUM")
        pp_o = tc.alloc_tile_pool(name="pp_o", bufs=2, space="PSUM")

        ones128 = const.tile([128, 128], F32R)
        nc.sync.dma_start(ones128[:], p_ones.ap())
        onesb = const.tile([128, 128], BF)
        nc.vector.tensor_copy(onesb[:], ones128[:])
        eps_t = const.tile([128, 1], F32)
        nc.gpsimd.memset(eps_t[:], EPS)
        ones64 = const.tile([1, 64], F32)
        nc.gpsimd.memset(ones64[:], 1.0)
        # diagonal-block masks [k, src, dup(head), i]
        m4 = const.tile([128, 2, 2, 128], BF)
        nc.sync.dma_start(m4[:], p_mask.ap().rearrange("s u k i -> k s u i"))

        # peer half index for dynamic landing offsets
        pid = nc.scalar.partition_id()
        peer = (pid + 1) % 2
        offK = peer * (128 * KW)
        offV = peer * (128 * VW)

        # residual stream h^T, f32, D-chunk d at [:, d, :]
        h = persist.tile([128, DCH, TH], F32R)
        # Q^T zero-padded per head: head hd's 64 dims live in rows
        # (hd%2)*64.. of [:, hd, :]; the other 64 rows stay zero.
        QT2 = persist.tile([128, NH, TH], BF)
        nc.gpsimd.memset(QT2[:], 0.0)
        yT = persist.tile([128, DCH, TH], BF)    # attn out^T, rows=D
        # own-rank K^T / packed V (staged to the peer); peer-rank landing
        kloc = persist.tile([128, DCH, TH], BF)
        vloc = persist.tile([128, VW], BF)
        KTp = persist.tile([128, DCH, TH], BF)
        VA2p = persist.tile([128, VW], BF)
        # ones columns of the V packing (vloc is gathered; VA2p receives
        # the peer's copy with its ones already in place)
        nc.gpsimd.memset(vloc[:], 0.0)
        nc.gpsimd.memset(
            vloc[:, 0:NH * VHB].rearrange("p (x o) -> p x o", o=65)[:, :, 64:65],
            1.0)

        # ---------------- helpers ----------------
        def layernorm(z_out, src):
            """z_out (sbuf bf16 [128, DCH, TH]) = LayerNorm(src) in D-major."""
            s_b = pp_mm.tile([128, TH], F32, tag="mm")
            for d in range(DCH):
                nc.tensor.matmul(s_b[:], ones128[:], src[:, d, :],
                                 start=(d == 0), stop=(d == DCH - 1))
            q_b = pp_mm.tile([128, TH], F32, tag="mm")
            for d in range(DCH):
                sq = tmp.tile([128, TH], BF, tag="sq", bufs=2)
                nc.scalar.square(sq[:], src[:, d, :])
                nc.tensor.matmul(q_b[:], onesb[:], sq[:],
                                 start=(d == 0), stop=(d == DCH - 1))
            ss = stat.tile([128, TH], F32, tag="stat")
            nc.scalar.square(ss[:], s_b[:])
            u = stat.tile([128, TH], F32, tag="stat")
            nc.vector.scalar_tensor_tensor(u[:], ss[:], -1.0 / D, q_b[:],
                                           AluOp.mult, AluOp.add)
            rinv = stat.tile([128, TH], F32, tag="stat")
            nc.scalar.activation(rinv[:], u[:], Act.Sqrt, scale=1.0 / D,
                                 bias=eps_t[:])
            nc.vector.reciprocal_approx_fast(rinv[:], rinv[:])
            mr = u
            nc.vector.scalar_tensor_tensor(mr[:], s_b[:], 1.0 / D, rinv[:],
                                           AluOp.mult, AluOp.mult)
            for d in range(DCH):
                t = tmp.tile([128, TH], F32, tag="zt", bufs=2)
                nc.gpsimd.tensor_tensor(t[:], src[:, d, :], rinv[:],
                                        AluOp.mult)
                nc.vector.tensor_tensor(z_out[:, d, :], t[:], mr[:],
                                        AluOp.subtract)

        # ---------------- embedding ----------------
        for r in range(4):
            wet = tmp.tile([128, 4, 128], BF, tag="tbf", bufs=2)
            nc.sync.dma_start(wet[:], p_we.ap()[r])
            ep = pp_mm.tile([128, TH], F32, tag="mm")
            for ec in range(4):
                lt = tmp.tile([128, TH], BF, tag="tbf", bufs=2)
                nc.sync.dma_start(lt[:], p_lcdT.ap()[ec * 128:(ec + 1) * 128, :])
                nc.tensor.matmul(ep[:], wet[:, ec, :], lt[:],
                                 start=(ec == 0), stop=(ec == 3))
            pt = tmp.tile([128, TH], F32, tag="t32", bufs=2)
            nc.sync.dma_start(pt[:], p_posT.ap()[r * 128:(r + 1) * 128, :])
            nc.vector.tensor_tensor(h[:, r, :], ep[:], pt[:], AluOp.add)
        actT = tmp.tile([AD, TH], F32, tag="t32", bufs=2)
        nc.sync.dma_start(actT[:], p_actT.ap())
        for r in range(4):
            wat = tmp.tile([AD, 128], F32, tag="t32", bufs=2)
            nc.sync.dma_start(wat[:], p_wa.ap()[:, r * 128:(r + 1) * 128])
            ap_ = pp_mm.tile([128, TH], F32, tag="mm")
            nc.tensor.matmul(ap_[:], wat[:], actT[:], start=True, stop=True)
            pt = tmp.tile([128, TH], F32, tag="t32", bufs=2)
            nc.sync.dma_start(pt[:], p_posT.ap()[(4 + r) * 128:(5 + r) * 128, :])
            nc.vector.tensor_tensor(h[:, 4 + r, :], ap_[:], pt[:], AluOp.add)

        # ---------------- transformer layers ----------------
        for l in range(NL):
            z1 = zpool.tile([128, DCH, TH], BF, tag="z", bufs=1)
            layernorm(z1, h)

            # ---- K^T own tokens -> kloc ----
            for r in range(DCH):
                wt = wpool.tile([128, DCH, 128], BF, tag="w")
                nc.sync.dma_start(wt[:], p_wk.ap()[l, r])
                kp = pp_mm.tile([128, TH], F32, tag="mm")
                for d in range(DCH):
                    nc.tensor.matmul(kp[:], wt[:, d, :], z1[:, d, :],
                                     start=(d == 0), stop=(d == DCH - 1))
                nc.vector.tensor_copy(kloc[:, r, :], kp[:])
            kin = dram.tile([128, KW], BF, tag="kin")
            nc.scalar.dma_start(kin[:], kloc[:])
            kout = dram.tile([2, 128, KW], BF, tag="kout")
            nc.gpsimd.collective_compute(
                "AllGather", AluOp.bypass,
                replica_groups=[[0, 1], [2, 3], [4, 5], [6, 7]],
                ins=[kin.opt()], outs=[kout.opt()])

            # ---- V own tokens, packed [V_h | ones] per head/block ----
            for nn in range(2):
                wvt = w8pool.tile([128, DCH, 512], BF, tag="w8")
                nc.sync.dma_start(wvt[:], p_wv.ap()[l, nn])
                for m in range(4):
                    vp = pp_mm.tile([128, 512], F32, tag="mm")
                    for d in range(DCH):
                        nc.tensor.matmul(
                            vp[:], z1[:, d, m * 128:(m + 1) * 128],
                            wvt[:, d, :],
                            start=(d == 0), stop=(d == DCH - 1))
                    nc.scalar.copy(
                        vloc[:, nn * 8 * VHB + m * 65:
                             nn * 8 * VHB + m * 65 + 8 * VHB].rearrange(
                            "p (hd x) -> p hd x", x=VHB)[:, :, 0:64],
                        vp.rearrange("p (hd dh) -> p hd dh", dh=DH))
            vin = dram.tile([128, VW], BF, tag="vin")
            nc.scalar.dma_start(vin[:], vloc[:])
            vout = dram.tile([2, 128, VW], BF, tag="vout")
            nc.gpsimd.collective_compute(
                "AllGather", AluOp.bypass,
                replica_groups=[[0, 1], [2, 3], [4, 5], [6, 7]],
                ins=[vin.opt()], outs=[vout.opt()])

            # ---- Q^T own tokens into zero-padded per-head slots ----
            for r in range(DCH):
                wt = wpool.tile([128, DCH, 128], BF, tag="w")
                nc.sync.dma_start(wt[:], p_wq.ap()[l, r])
                qp = pp_mm.tile([128, TH], F32, tag="mm")
                for d in range(DCH):
                    nc.tensor.matmul(qp[:], wt[:, d, :], z1[:, d, :],
                                     start=(d == 0), stop=(d == DCH - 1))
                nc.vector.tensor_copy(QT2[0:64, 2 * r, :], qp[0:64, :])
                nc.vector.tensor_copy(QT2[64:128, 2 * r + 1, :], qp[64:128, :])

            # ---- land the PEER half of the gathers (dynamic offset) ----
            kb = kout[0].rearrange("p (d t) -> p d t", t=TH)
            nc.scalar.dma_start(KTp[:], dataclasses.replace(
                kb, offset=kb.offset + offK, dep_tracking_offset=kb.offset))
            vb = vout[0]
            nc.scalar.dma_start(VA2p[:], dataclasses.replace(
                vb, offset=vb.offset + offV, dep_tracking_offset=vb.offset))

            # ---- attention ----
            for hd in range(NH):
                rc, ro = hd // 2, (hd % 2) * 64
                o_p = pp_o.tile([128, TH], F32, tag="o")
                p2s = []
                for m in range(4):
                    W = TH - m * 128
                    s2 = pp_pair.tile([128, 2, TH], F32, tag="pair")
                    for si, srcK in ((0, kloc), (1, KTp)):
                        nc.tensor.matmul(
                            s2[:, si, 0:W],
                            srcK[:, rc, m * 128:(m + 1) * 128],
                            QT2[:, hd, m * 128:TH],
                            start=True, stop=True)
                    p2 = ptp.tile([128, 2, TH], BF, tag="p2")
                    nc.scalar.activation(p2[:, :, 0:W], s2[:, :, 0:W],
                                         Act.Exp,
                                         scale=1.0 / float(np.sqrt(DH)))
                    nc.gpsimd.tensor_tensor(p2[:, :, 0:128],
                                            p2[:, :, 0:128],
                                            m4[:, :, 0, :], AluOp.mult)
                    p2s.append(p2)
                for m in range(4):
                    W = TH - m * 128
                    for si, srcV in ((0, vloc), (1, VA2p)):
                        nc.tensor.matmul(
                            o_p[:, m * 128:TH],
                            srcV[:, hd * VHB + m * 65:
                                 hd * VHB + m * 65 + 128],
                            p2s[m][:, si, 0:W],
                            start=(m == 0 and si == 0),
                            stop=(m == 3 and si == 1))
                inv1 = stat.tile([1, TH], F32, tag="den1", bufs=1)
                nc.vector.tensor_copy(inv1[:], o_p[64:65, :])
                nc.vector.reciprocal_approx_fast(inv1[:], inv1[:])
                invO = stat.tile([64, TH], F32, tag="den", bufs=1)
                nc.gpsimd.partition_broadcast(invO[:], inv1[0:1, :])
                nc.vector.tensor_tensor(yT[ro:ro + 64, rc, :], o_p[0:64, :],
                                        invO[:], AluOp.mult)

            # ---- proj + residual ----
            for r in range(DCH):
                wt = wpool.tile([128, DCH, 128], BF, tag="w")
                nc.sync.dma_start(wt[:], p_wp.ap()[l, r])
                pp = pp_mm.tile([128, TH], F32, tag="mm")
                for d in range(DCH):
                    nc.tensor.matmul(pp[:], wt[:, d, :], yT[:, d, :],
                                     start=(d == 0), stop=(d == DCH - 1))
                nc.vector.tensor_tensor(h[:, r, :], h[:, r, :], pp[:],
                                        AluOp.add)

            # ---- MLP ----
            z2 = zpool.tile([128, DCH, TH], BF, tag="z", bufs=1)
            layernorm(z2, h)
            aT = big.tile([128, 32, TH], BF, tag="aT")
            w2ts = []
            for r in range(2):
                w2t = w8pool.tile([128, FF // 128, 128], BF, tag="w8")
                nc.scalar.dma_start(w2t[:], p_w2.ap()[l, r])
                w2ts.append(w2t)
            for fi in range(16):
                w1t = w1pool.tile([128, DCH, 256], BF, tag="w1p")
                nc.sync.dma_start(w1t[:], p_w1.ap()[l, fi])
                fp = pp_pair.tile([128, 2, TH], F32, tag="pair")
                for j in range(2):
                    for d in range(DCH):
                        nc.tensor.matmul(
                            fp[:, j, :], w1t[:, d, j * 128:(j + 1) * 128],
                            z2[:, d, :],
                            start=(d == 0), stop=(d == DCH - 1))
                nc.scalar.activation(aT[:, 2 * fi:2 * fi + 2, :], fp[:],
                                     Act.Gelu)
            for r in range(DCH):
                if r + 2 < DCH:
                    w2t = w8pool.tile([128, FF // 128, 128], BF, tag="w8")
                    nc.scalar.dma_start(w2t[:], p_w2.ap()[l, r + 2])
                    w2ts.append(w2t)
                w2t = w2ts[r]
                mp = pp_mm.tile([128, TH], F32, tag="mm")
                for fc in range(32):
                    nc.tensor.matmul(mp[:], w2t[:, fc, :], aT[:, fc, :],
                                     start=(fc == 0), stop=(fc == 31))
                nc.vector.tensor_tensor(h[:, r, :], h[:, r, :], mp[:],
                                        AluOp.add)

        # ---------------- final LN + head ----------------
        zf = zpool.tile([128, DCH, TH], BF, tag="z", bufs=1)
        layernorm(zf, h)
        wht = whpool.tile([128, DCH, E], BF, tag="wh")
        nc.sync.dma_start(wht[:], p_wh.ap())
        for tb in range(4):
            op_ = pp_mm.tile([128, E], F32, tag="mm")
            for d in range(DCH):
                nc.tensor.matmul(
                    op_[:],
                    zf[:, d, tb * 128:(tb + 1) * 128],
                    wht[:, d, :],
                    start=(d == 0), stop=(d == DCH - 1))
            ot = tmp.tile([128, E], F32, tag="t32", bufs=2)
            nc.scalar.copy(ot[:], op_[:])
            nc.sync.dma_start(p_out.ap()[tb * 128:(tb + 1) * 128, :], ot[:])

        for _pool in reversed((const, persist, zpool, big, wpool, w1pool,
                               w8pool, whpool, tmp, stat, ptp, dram, pp_mm,
                               pp_pair, pp_o)):
            _pool.release()

    nc.compile()
    return nc


def _get_program():
    if "nc" not in _cache:
        _cache["nc"] = _build_program()
    return _cache["nc"]


def _bf16(x):
    return np.ascontiguousarray(np.asarray(x).astype(ml_dtypes.bfloat16))


def _f32(x):
    return np.ascontiguousarray(np.asarray(x).astype(np.float32))


def _pack_sq(w):
    """[Din,Dout] -> [r, p, dc, c] = w[dc*128+p, r*128+c] per-r contiguous."""
    din, dout = w.shape
    return w.reshape(din // 128, 128, dout // 128, 128).transpose(2, 1, 0, 3)


def _pack_w(w, cols):
    """[Din,Dout] -> [i, p, dc, cols] = w[dc*128+p, i*cols+c]."""
    din, dout = w.shape
    return w.reshape(din // 128, 128, dout // cols, cols).transpose(2, 1, 0, 3)


def make_in_maps(inputs):
    lcd = np.asarray(inputs["lcd"], np.float32).reshape(B, T, E)
    lcd_shift = np.concatenate(
        [np.zeros((B, 1, E), np.float32), lcd[:, :-1]], axis=1)
    action = np.asarray(inputs["action"], np.float32)
    pos = np.asarray(inputs["pos_emb"], np.float32)[0]          # [T, D]

    shared = {
        "W_embed": _bf16(_pack_w(np.asarray(inputs["W_embed"]), 128)),
        "W_act": _f32(inputs["W_act"]),
        "Wq": _bf16(np.stack([_pack_sq(w) for w in np.asarray(inputs["Wq"])])),
        "Wk": _bf16(np.stack([_pack_sq(w) for w in np.asarray(inputs["Wk"])])),
        "Wv": _bf16(np.stack([_pack_w(w, 512) for w in np.asarray(inputs["Wv"])])),
        "Wp": _bf16(np.stack([_pack_sq(w) for w in np.asarray(inputs["Wp"])])),
        "W1": _bf16(np.stack([_pack_w(w, 256) for w in np.asarray(inputs["W1"])])),
        "W2": _bf16(np.stack([_pack_sq(w) for w in np.asarray(inputs["W2"])])),
        "Wh": _bf16(np.asarray(inputs["Wh"]).reshape(8, 128, E).transpose(1, 0, 2)),
        "ones128": np.ones((128, 128), np.float32),
    }

    kk, ii = np.arange(128)[:, None], np.arange(128)[None, :]
    in_maps = []
    for c in range(NC):
        b, r = c // 2, c % 2
        tok = np.arange(T)[r::2]                                # own tokens
        # diagonal-block masks [src(own/peer), dup, k, i]:
        # own: k <= i; peer: 2k + (1-r) <= 2i + r
        mown = (kk <= ii).astype(np.float32)
        mpeer = (2 * kk + (1 - r) <= 2 * ii + r).astype(np.float32)
        mask4 = np.stack([np.stack([mown, mown]), np.stack([mpeer, mpeer])])
        in_maps.append(dict(
            shared,
            lcdT=_bf16(lcd_shift[b, tok].T),                    # [E, TH]
            actT=_f32(action[b, tok].T),                        # [AD, TH]
            posT=_f32(pos[tok].T),                              # [D, TH]
            mask4=_bf16(mask4),
        ))
    return in_maps


def assemble(results):
    out = np.empty((B, T, E), np.float32)
    for c in range(NC):
        b, r = c // 2, c % 2
        out[b, r::2] = results[c]["out"]
    return out


def kernel(**inputs):
    nc = _get_program()
    in_maps = make_in_maps(inputs)
    res = run_bass_kernel_spmd(nc, in_maps, list(range(NC)))
    return assemble(res.results)


# revision 32
# speedup vs baseline: 1.6771x; 1.6771x over previous
"""GPT forward pass on 8 TRN2 NeuronCores.

Sharding: core c -> batch b = c // 2, rank r = c % 2 owns tokens t with
t % 2 == r (even/odd interleave of the sequence).  The residual stream is
core-local in D-major layout (h^T: [D partition-chunks, 512 own tokens]).

Attention uses rank-pure key blocks: key block (src, m) holds one rank's
local keys 128m..128m+127 (own rank: src=0 from local kloc/vloc; peer
rank: src=1 from the gathered KTp/VA2p).  Causality at 128-block
granularity is uniform across cores: query block j needs key blocks
m <= j of both ranks, with the diagonal m == j masked by a data-driven
[2,128,128] 0/1 mask (own: k<=i; peer: r=0 -> k<i, r=1 -> k<=i).  This
cuts S/PV work from 2*32 to 2*20 matmul units per head per layer.
S matmuls are grouped per key block with suffix moving slices
(widths 512,384,256,128 per rank); PV accumulates the same suffixes
into one PSUM bank per head, with a trailing ones column in the packed
V producing the softmax denominator in PSUM row 64.

Per layer two AllGathers over the core pair exchange K^T and the
PV-packed V ([V_h | ones] blocks); only the PEER half is landed, via a
partition_id()-derived dynamic DRAM offset.  Q runs between the gather
issue and the first peer-dependent matmul.

Weights are pre-packed on the host so every weight DMA is
partition-major contiguous (2-8KB per partition per transfer).
"""

import sys

sys.path.insert(0, "/opt/trn_rl_repo")

import dataclasses
import numpy as np
import ml_dtypes

import concourse.bass as bass
import concourse.bacc as bacc
import concourse.mybir as mybir
from concourse import tile
from concourse.bass_utils import run_bass_kernel_spmd

B, T, E, D, NH, DH, NL, FF, AD = 4, 1024, 512, 1024, 16, 64, 8, 4096, 8
TH = T // 2          # tokens per core
NC = 8
DCH = D // 128       # 8 partition chunks of the embedding dim
EPS = 1e-5
BF = mybir.dt.bfloat16
F32 = mybir.dt.float32
F32R = mybir.dt.float32r
AluOp = mybir.AluOpType
Act = mybir.ActivationFunctionType

# V packing: per rank, per head: 4 key blocks of [V_h(64) | ones(1)];
# the 128-wide PV stationary slice reads up to 63 junk cols past block 3.
VHB = 4 * 65                 # 260 cols per head
VW = NH * VHB + 200          # 4360 cols per rank (pad for copy/read APs)
KW = DCH * TH                # 4096 cols of K^T per rank

_cache = {}


def _build_program():
    nc = bacc.Bacc("TRN2", target_bir_lowering=False, debug=False, num_devices=NC)

    # --- DRAM parameters (identical graph on all cores; data differs) ---
    p_lcdT = nc.declare_dram_parameter("lcdT", [E, TH], BF, isOutput=False)
    p_actT = nc.declare_dram_parameter("actT", [AD, TH], F32, isOutput=False)
    p_posT = nc.declare_dram_parameter("posT", [D, TH], F32, isOutput=False)
    p_we = nc.declare_dram_parameter("W_embed", [4, 128, 4, 128], BF, isOutput=False)
    p_wa = nc.declare_dram_parameter("W_act", [AD, D // 2], F32, isOutput=False)
    p_wq = nc.declare_dram_parameter("Wq", [NL, 8, 128, 8, 128], BF, isOutput=False)
    p_wk = nc.declare_dram_parameter("Wk", [NL, 8, 128, 8, 128], BF, isOutput=False)
    p_wv = nc.declare_dram_parameter("Wv", [NL, 2, 128, 8, 512], BF, isOutput=False)
    p_wp = nc.declare_dram_parameter("Wp", [NL, 8, 128, 8, 128], BF, isOutput=False)
    p_w1 = nc.declare_dram_parameter("W1", [NL, 16, 128, 8, 256], BF, isOutput=False)
    p_w2 = nc.declare_dram_parameter("W2", [NL, 8, 128, 32, 128], BF, isOutput=False)
    p_wh = nc.declare_dram_parameter("Wh", [128, 8, E], BF, isOutput=False)
    p_mask = nc.declare_dram_parameter("mask4", [2, 2, 128, 128], BF, isOutput=False)
    p_ones = nc.declare_dram_parameter("ones128", [128, 128], F32R, isOutput=False)
    p_out = nc.declare_dram_parameter("out", [TH, E], F32, isOutput=True)

    with tile.TileContext(nc) as tc:
        # ---------------- pools ----------------
        const = tc.alloc_tile_pool(name="const", bufs=1)
        persist = tc.alloc_tile_pool(name="persist", bufs=1)
        zpool = tc.alloc_tile_pool(name="zpool", bufs=1)
        big = tc.alloc_tile_pool(name="bigact", bufs=1)
        wpool = tc.alloc_tile_pool(name="wpool", bufs=6)
        w1pool = tc.alloc_tile_pool(name="w1pool", bufs=3)
        w8pool = tc.alloc_tile_pool(name="w8pool", bufs=3)
        whpool = tc.alloc_tile_pool(name="whpool", bufs=1)
        tmp = tc.alloc_tile_pool(name="tmp", bufs=3)
        stat = tc.alloc_tile_pool(name="stat", bufs=3)
        ptp = tc.alloc_tile_pool(name="ptp", bufs=6)
        dram = tc.alloc_tile_pool(name="dram", bufs=2, space="DRAM")
        pp_mm = tc.alloc_tile_pool(name="pp_mm", bufs=2, space="PSUM")
        pp_pair = tc.alloc_tile_pool(name="pp_pair", bufs=2, space="PSUM")
        pp_o = tc.alloc_tile_pool(name="pp_o", bufs=2, space="PSUM")

        ones128 = const.tile([128, 128], F32R)
        nc.sync.dma_start(ones128[:], p_ones.ap())
        onesb = const.tile([128, 128], BF)
        nc.vector.tensor_copy(onesb[:], ones128[:])
        eps_t = const.tile([128, 1], F32)
        nc.gpsimd.memset(eps_t[:], EPS)
        ones64 = const.tile([1, 64], F32)
        nc.gpsimd.memset(ones64[:], 1.0)
        # diagonal-block masks [k, src, dup(head), i]
        m4 = const.tile([128, 2, 2, 128], BF)
        nc.sync.dma_start(m4[:], p_mask.ap().rearrange("s u k i -> k s u i"))

        # peer half index for dynamic landing offsets
        pid = nc.scalar.partition_id()
        peer = (pid + 1) % 2
        offK = peer * (128 * KW)
        offV = peer * (128 * VW)

        # residual stream h^T, f32, D-chunk d at [:, d, :]
        h = persist.tile([128, DCH, TH], F32R)
        # Q^T zero-padded per head: head hd's 64 dims live in rows
        # (hd%2)*64.. of [:, hd, :]; the other 64 rows stay zero.
        QT2 = persist.tile([128, NH, TH], BF)
        nc.gpsimd.memset(QT2[:], 0.0)
        yT = persist.tile([128, DCH, TH], BF)    # attn out^T, rows=D
        # own-rank K^T / packed V (staged to the peer); peer-rank landing
        kloc = persist.tile([128, DCH, TH], BF)
        vloc = persist.tile([128, VW], BF)
        KTp = persist.tile([128, DCH, TH], BF)
        VA2p = persist.tile([128, VW], BF)
        # ones columns of the V packing (vloc is gathered; VA2p receives
        # the peer's copy with its ones already in place)
        nc.gpsimd.memset(vloc[:], 0.0)
        nc.gpsimd.memset(
            vloc[:, 0:NH * VHB].rearrange("p (x o) -> p x o", o=65)[:, :, 64:65],
            1.0)

        # ---------------- helpers ----------------
        def layernorm(z_out, src):
            """z_out (sbuf bf16 [128, DCH, TH]) = LayerNorm(src) in D-major."""
            s_b = pp_mm.tile([128, TH], F32, tag="mm")
            for d in range(DCH):
                nc.tensor.matmul(s_b[:], ones128[:], src[:, d, :],
                                 start=(d == 0), stop=(d == DCH - 1))
            q_b = pp_mm.tile([128, TH], F32, tag="mm")
            for d in range(DCH):
                sq = tmp.tile([128, TH], BF, tag="sq", bufs=2)
                nc.scalar.square(sq[:], src[:, d, :])
                nc.tensor.matmul(q_b[:], onesb[:], sq[:],
                                 start=(d == 0), stop=(d == DCH - 1))
            ss = stat.tile([128, TH], F32, tag="stat")
            nc.scalar.square(ss[:], s_b[:])
            u = stat.tile([128, TH], F32, tag="stat")
            nc.vector.scalar_tensor_tensor(u[:], ss[:], -1.0 / D, q_b[:],
                                           AluOp.mult, AluOp.add)
            rinv = stat.tile([128, TH], F32, tag="stat")
            nc.scalar.activation(rinv[:], u[:], Act.Sqrt, scale=1.0 / D,
                                 bias=eps_t[:])
            nc.vector.reciprocal_approx_fast(rinv[:], rinv[:])
            mr = u
            nc.vector.scalar_tensor_tensor(mr[:], s_b[:], 1.0 / D, rinv[:],
                                           AluOp.mult, AluOp.mult)
            for d in range(DCH):
                t = tmp.tile([128, TH], F32, tag="zt", bufs=2)
                nc.vector.tensor_tensor(t[:], src[:, d, :], rinv[:],
                                        AluOp.mult)
                nc.vector.tensor_tensor(z_out[:, d, :], t[:], mr[:],
                                        AluOp.subtract)

        # ---------------- embedding ----------------
        for r in range(4):
            wet = tmp.tile([128, 4, 128], BF, tag="tbf", bufs=2)
            nc.sync.dma_start(wet[:], p_we.ap()[r])
            ep = pp_mm.tile([128, TH], F32, tag="mm")
            for ec in range(4):
                lt = tmp.tile([128, TH], BF, tag="tbf", bufs=2)
                nc.sync.dma_start(lt[:], p_lcdT.ap()[ec * 128:(ec + 1) * 128, :])
                nc.tensor.matmul(ep[:], wet[:, ec, :], lt[:],
                                 start=(ec == 0), stop=(ec == 3))
            pt = tmp.tile([128, TH], F32, tag="t32", bufs=2)
            nc.sync.dma_start(pt[:], p_posT.ap()[r * 128:(r + 1) * 128, :])
            nc.vector.tensor_tensor(h[:, r, :], ep[:], pt[:], AluOp.add)
        actT = tmp.tile([AD, TH], F32, tag="t32", bufs=2)
        nc.sync.dma_start(actT[:], p_actT.ap())
        for r in range(4):
            wat = tmp.tile([AD, 128], F32, tag="t32", bufs=2)
            nc.sync.dma_start(wat[:], p_wa.ap()[:, r * 128:(r + 1) * 128])
            ap_ = pp_mm.tile([128, TH], F32, tag="mm")
            nc.tensor.matmul(ap_[:], wat[:], actT[:], start=True, stop=True)
            pt = tmp.tile([128, TH], F32, tag="t32", bufs=2)
            nc.sync.dma_start(pt[:], p_posT.ap()[(4 + r) * 128:(5 + r) * 128, :])
            nc.vector.tensor_tensor(h[:, 4 + r, :], ap_[:], pt[:], AluOp.add)

        # ---------------- transformer layers ----------------
        for l in range(NL):
            z1 = zpool.tile([128, DCH, TH], BF, tag="z", bufs=1)
            layernorm(z1, h)

            # ---- K^T own tokens -> kloc ----
            for r in range(DCH):
                wt = wpool.tile([128, DCH, 128], BF, tag="w")
                nc.sync.dma_start(wt[:], p_wk.ap()[l, r])
                kp = pp_mm.tile([128, TH], F32, tag="mm")
                for d in range(DCH):
                    nc.tensor.matmul(kp[:], wt[:, d, :], z1[:, d, :],
                                     start=(d == 0), stop=(d == DCH - 1))
                nc.vector.tensor_copy(kloc[:, r, :], kp[:])
            kin = dram.tile([128, KW], BF, tag="kin")
            nc.scalar.dma_start(kin[:], kloc[:])
            kout = dram.tile([2, 128, KW], BF, tag="kout")
            nc.gpsimd.collective_compute(
                "AllGather", AluOp.bypass,
                replica_groups=[[0, 1], [2, 3], [4, 5], [6, 7]],
                ins=[kin.opt()], outs=[kout.opt()])

            # ---- V own tokens, packed [V_h | ones] per head/block ----
            for nn in range(2):
                wvt = w8pool.tile([128, DCH, 512], BF, tag="w8")
                nc.sync.dma_start(wvt[:], p_wv.ap()[l, nn])
                for m in range(4):
                    vp = pp_mm.tile([128, 512], F32, tag="mm")
                    for d in range(DCH):
                        nc.tensor.matmul(
                            vp[:], z1[:, d, m * 128:(m + 1) * 128],
                            wvt[:, d, :],
                            start=(d == 0), stop=(d == DCH - 1))
                    nc.scalar.copy(
                        vloc[:, nn * 8 * VHB + m * 65:
                             nn * 8 * VHB + m * 65 + 8 * VHB].rearrange(
                            "p (hd x) -> p hd x", x=VHB)[:, :, 0:64],
                        vp.rearrange("p (hd dh) -> p hd dh", dh=DH))
            vin = dram.tile([128, VW], BF, tag="vin")
            nc.scalar.dma_start(vin[:], vloc[:])
            vout = dram.tile([2, 128, VW], BF, tag="vout")
            nc.gpsimd.collective_compute(
                "AllGather", AluOp.bypass,
                replica_groups=[[0, 1], [2, 3], [4, 5], [6, 7]],
                ins=[vin.opt()], outs=[vout.opt()])

            # ---- Q^T own tokens into zero-padded per-head slots ----
            for r in range(DCH):
                wt = wpool.tile([128, DCH, 128], BF, tag="w")
                nc.sync.dma_start(wt[:], p_wq.ap()[l, r])
                qp = pp_mm.tile([128, TH], F32, tag="mm")
                for d in range(DCH):
                    nc.tensor.matmul(qp[:], wt[:, d, :], z1[:, d, :],
                                     start=(d == 0), stop=(d == DCH - 1))
                nc.vector.tensor_copy(QT2[0:64, 2 * r, :], qp[0:64, :])
                nc.vector.tensor_copy(QT2[64:128, 2 * r + 1, :], qp[64:128, :])

            # ---- land the PEER half of the gathers (dynamic offset) ----
            kb = kout[0].rearrange("p (d t) -> p d t", t=TH)
            nc.scalar.dma_start(KTp[:], dataclasses.replace(
                kb, offset=kb.offset + offK, dep_tracking_offset=kb.offset))
            vb = vout[0]
            nc.scalar.dma_start(VA2p[:], dataclasses.replace(
                vb, offset=vb.offset + offV, dep_tracking_offset=vb.offset))

            # ---- attention ----
            for hd in range(NH):
                rc, ro = hd // 2, (hd % 2) * 64
                o_p = pp_o.tile([128, TH], F32, tag="o")
                p2s = []
                for m in range(4):
                    W = TH - m * 128
                    s2 = pp_pair.tile([128, 2, TH], F32, tag="pair")
                    for si, srcK in ((0, kloc), (1, KTp)):
                        nc.tensor.matmul(
                            s2[:, si, 0:W],
                            srcK[:, rc, m * 128:(m + 1) * 128],
                            QT2[:, hd, m * 128:TH],
                            start=True, stop=True)
                    p2 = ptp.tile([128, 2, TH], BF, tag="p2")
                    nc.scalar.activation(p2[:, :, 0:W], s2[:, :, 0:W],
                                         Act.Exp,
                                         scale=1.0 / float(np.sqrt(DH)))
                    nc.vector.tensor_tensor(p2[:, :, 0:128],
                                            p2[:, :, 0:128],
                                            m4[:, :, 0, :], AluOp.mult)
                    p2s.append(p2)
                for m in range(4):
                    W = TH - m * 128
                    for si, srcV in ((0, vloc), (1, VA2p)):
                        nc.tensor.matmul(
                            o_p[:, m * 128:TH],
                            srcV[:, hd * VHB + m * 65:
                                 hd * VHB + m * 65 + 128],
                            p2s[m][:, si, 0:W],
                            start=(m == 0 and si == 0),
                            stop=(m == 3 and si == 1))
                inv1 = stat.tile([1, TH], F32, tag="den1", bufs=1)
                nc.vector.tensor_copy(inv1[:], o_p[64:65, :])
                nc.vector.reciprocal_approx_fast(inv1[:], inv1[:])
                invO = stat.tile([64, TH], F32, tag="den", bufs=1)
                nc.gpsimd.partition_broadcast(invO[:], inv1[0:1, :])
                nc.vector.tensor_tensor(yT[ro:ro + 64, rc, :], o_p[0:64, :],
                                        invO[:], AluOp.mult)

            # ---- proj + residual ----
            for r in range(DCH):
                wt = wpool.tile([128, DCH, 128], BF, tag="w")
                nc.sync.dma_start(wt[:], p_wp.ap()[l, r])
                pp = pp_mm.tile([128, TH], F32, tag="mm")
                for d in range(DCH):
                    nc.tensor.matmul(pp[:], wt[:, d, :], yT[:, d, :],
                                     start=(d == 0), stop=(d == DCH - 1))
                nc.vector.tensor_tensor(h[:, r, :], h[:, r, :], pp[:],
                                        AluOp.add)

            # ---- MLP ----
            z2 = zpool.tile([128, DCH, TH], BF, tag="z", bufs=1)
            layernorm(z2, h)
            aT = big.tile([128, 32, TH], BF, tag="aT")
            w2ts = []
            for r in range(2):
                w2t = w8pool.tile([128, FF // 128, 128], BF, tag="w8")
                nc.scalar.dma_start(w2t[:], p_w2.ap()[l, r])
                w2ts.append(w2t)
            for fi in range(16):
                w1t = w1pool.tile([128, DCH, 256], BF, tag="w1p")
                nc.sync.dma_start(w1t[:], p_w1.ap()[l, fi])
                fp = pp_pair.tile([128, 2, TH], F32, tag="pair")
                for j in range(2):
                    for d in range(DCH):
                        nc.tensor.matmul(
                            fp[:, j, :], w1t[:, d, j * 128:(j + 1) * 128],
                            z2[:, d, :],
                            start=(d == 0), stop=(d == DCH - 1))
                nc.scalar.activation(aT[:, 2 * fi:2 * fi + 2, :], fp[:],
                                     Act.Gelu)
            for r in range(DCH):
                if r + 2 < DCH:
                    w2t = w8pool.tile([128, FF // 128, 128], BF, tag="w8")
                    nc.scalar.dma_start(w2t[:], p_w2.ap()[l, r + 2])
                    w2ts.append(w2t)
                w2t = w2ts[r]
                mp = pp_mm.tile([128, TH], F32, tag="mm")
                for fc in range(32):
                    nc.tensor.matmul(mp[:], w2t[:, fc, :], aT[:, fc, :],
                                     start=(fc == 0), stop=(fc == 31))
                nc.vector.tensor_tensor(h[:, r, :], h[:, r, :], mp[:],
                                        AluOp.add)

        # ---------------- final LN + head ----------------
        zf = zpool.tile([128, DCH, TH], BF, tag="z", bufs=1)
        layernorm(zf, h)
        wht = whpool.tile([128, DCH, E], BF, tag="wh")
        nc.sync.dma_start(wht[:], p_wh.ap())
        for tb in range(4):
            op_ = pp_mm.tile([128, E], F32, tag="mm")
            for d in range(DCH):
                nc.tensor.matmul(
                    op_[:],
                    zf[:, d, tb * 128:(tb + 1) * 128],
                    wht[:, d, :],
                    start=(d == 0), stop=(d == DCH - 1))
            ot = tmp.tile([128, E], F32, tag="t32", bufs=2)
            nc.scalar.copy(ot[:], op_[:])
            nc.sync.dma_start(p_out.ap()[tb * 128:(tb + 1) * 128, :], ot[:])

        for _pool in reversed((const, persist, zpool, big, wpool, w1pool,
                               w8pool, whpool, tmp, stat, ptp, dram, pp_mm,
                               pp_pair, pp_o)):
            _pool.release()

    nc.compile()
    return nc


def _get_program():
    if "nc" not in _cache:
        _cache["nc"] = _build_program()
    return _cache["nc"]


def _bf16(x):
    return np.ascontiguousarray(np.asarray(x).astype(ml_dtypes.bfloat16))


def _f32(x):
    return np.ascontiguousarray(np.asarray(x).astype(np.float32))


def _pack_sq(w):
    """[Din,Dout] -> [r, p, dc, c] = w[dc*128+p, r*128+c] per-r contiguous."""
    din, dout = w.shape
    return w.reshape(din // 128, 128, dout // 128, 128).transpose(2, 1, 0, 3)


def _pack_w(w, cols):
    """[Din,Dout] -> [i, p, dc, cols] = w[dc*128+p, i*cols+c]."""
    din, dout = w.shape
    return w.reshape(din // 128, 128, dout // cols, cols).transpose(2, 1, 0, 3)


def make_in_maps(inputs):
    lcd = np.asarray(inputs["lcd"], np.float32).reshape(B, T, E)
    lcd_shift = np.concatenate(
        [np.zeros((B, 1, E), np.float32), lcd[:, :-1]], axis=1)
    action = np.asarray(inputs["action"], np.float32)
    pos = np.asarray(inputs["pos_emb"], np.float32)[0]          # [T, D]

    shared = {
        "W_embed": _bf16(_pack_w(np.asarray(inputs["W_embed"]), 128)),
        "W_act": _f32(inputs["W_act"]),
        "Wq": _bf16(np.stack([_pack_sq(w) for w in np.asarray(inputs["Wq"])])),
        "Wk": _bf16(np.stack([_pack_sq(w) for w in np.asarray(inputs["Wk"])])),
        "Wv": _bf16(np.stack([_pack_w(w, 512) for w in np.asarray(inputs["Wv"])])),
        "Wp": _bf16(np.stack([_pack_sq(w) for w in np.asarray(inputs["Wp"])])),
        "W1": _bf16(np.stack([_pack_w(w, 256) for w in np.asarray(inputs["W1"])])),
        "W2": _bf16(np.stack([_pack_sq(w) for w in np.asarray(inputs["W2"])])),
        "Wh": _bf16(np.asarray(inputs["Wh"]).reshape(8, 128, E).transpose(1, 0, 2)),
        "ones128": np.ones((128, 128), np.float32),
    }

    kk, ii = np.arange(128)[:, None], np.arange(128)[None, :]
    in_maps = []
    for c in range(NC):
        b, r = c // 2, c % 2
        tok = np.arange(T)[r::2]                                # own tokens
        # diagonal-block masks [src(own/peer), dup, k, i]:
        # own: k <= i; peer: 2k + (1-r) <= 2i + r
        mown = (kk <= ii).astype(np.float32)
        mpeer = (2 * kk + (1 - r) <= 2 * ii + r).astype(np.float32)
        mask4 = np.stack([np.stack([mown, mown]), np.stack([mpeer, mpeer])])
        in_maps.append(dict(
            shared,
            lcdT=_bf16(lcd_shift[b, tok].T),                    # [E, TH]
            actT=_f32(action[b, tok].T),                        # [AD, TH]
            posT=_f32(pos[tok].T),                              # [D, TH]
            mask4=_bf16(mask4),
        ))
    return in_maps


def assemble(results):
    out = np.empty((B, T, E), np.float32)
    for c in range(NC):
        b, r = c // 2, c % 2
        out[b, r::2] = results[c]["out"]
    return out


def kernel(**inputs):
    nc = _get_program()
    in_maps = make_in_maps(inputs)
    res = run_bass_kernel_spmd(nc, in_maps, list(range(NC)))
    return assemble(res.results)


# revision 33
# speedup vs baseline: 1.6956x; 1.0110x over previous
"""GPT forward pass on 8 TRN2 NeuronCores.

Sharding: core c -> batch b = c // 2, rank r = c % 2 owns tokens t with
t % 2 == r (even/odd interleave of the sequence).  The residual stream is
core-local in D-major layout (h^T: [D partition-chunks, 512 own tokens]).

Attention uses rank-pure key blocks: key block (src, m) holds one rank's
local keys 128m..128m+127 (own rank: src=0 from local kloc/vloc; peer
rank: src=1 from the gathered KTp/VA2p).  Causality at 128-block
granularity is uniform across cores: query block j needs key blocks
m <= j of both ranks, with the diagonal m == j masked by a data-driven
[2,128,128] 0/1 mask (own: k<=i; peer: r=0 -> k<i, r=1 -> k<=i).  This
cuts S/PV work from 2*32 to 2*20 matmul units per head per layer.
S matmuls are grouped per key block with suffix moving slices
(widths 512,384,256,128 per rank); PV accumulates the same suffixes
into one PSUM bank per head, with a trailing ones column in the packed
V producing the softmax denominator in PSUM row 64.

Per layer two AllGathers over the core pair exchange K^T and the
PV-packed V ([V_h | ones] blocks); only the PEER half is landed, via a
partition_id()-derived dynamic DRAM offset.  Q runs between the gather
issue and the first peer-dependent matmul.

Weights are pre-packed on the host so every weight DMA is
partition-major contiguous (2-8KB per partition per transfer).
"""

import sys

sys.path.insert(0, "/opt/trn_rl_repo")

import dataclasses
import numpy as np
import ml_dtypes

import concourse.bass as bass
import concourse.bacc as bacc
import concourse.mybir as mybir
from concourse import tile
from concourse.bass_utils import run_bass_kernel_spmd

B, T, E, D, NH, DH, NL, FF, AD = 4, 1024, 512, 1024, 16, 64, 8, 4096, 8
TH = T // 2          # tokens per core
NC = 8
DCH = D // 128       # 8 partition chunks of the embedding dim
EPS = 1e-5
BF = mybir.dt.bfloat16
F32 = mybir.dt.float32
F32R = mybir.dt.float32r
AluOp = mybir.AluOpType
Act = mybir.ActivationFunctionType

# V packing: per rank, per head: 4 key blocks of [V_h(64) | ones(1)];
# the 128-wide PV stationary slice reads up to 63 junk cols past block 3.
VHB = 4 * 65                 # 260 cols per head
VW = NH * VHB + 200          # 4360 cols per rank (pad for copy/read APs)
KW = DCH * TH                # 4096 cols of K^T per rank

_cache = {}


def _build_program():
    nc = bacc.Bacc("TRN2", target_bir_lowering=False, debug=False, num_devices=NC)

    # --- DRAM parameters (identical graph on all cores; data differs) ---
    p_lcdT = nc.declare_dram_parameter("lcdT", [E, TH], BF, isOutput=False)
    p_actT = nc.declare_dram_parameter("actT", [AD, TH], F32, isOutput=False)
    p_posT = nc.declare_dram_parameter("posT", [D, TH], F32, isOutput=False)
    p_we = nc.declare_dram_parameter("W_embed", [4, 128, 4, 128], BF, isOutput=False)
    p_wa = nc.declare_dram_parameter("W_act", [AD, D // 2], F32, isOutput=False)
    p_wq = nc.declare_dram_parameter("Wq", [NL, 8, 128, 8, 128], BF, isOutput=False)
    p_wk = nc.declare_dram_parameter("Wk", [NL, 8, 128, 8, 128], BF, isOutput=False)
    p_wv = nc.declare_dram_parameter("Wv", [NL, 2, 128, 8, 512], BF, isOutput=False)
    p_wp = nc.declare_dram_parameter("Wp", [NL, 8, 128, 8, 128], BF, isOutput=False)
    p_w1 = nc.declare_dram_parameter("W1", [NL, 16, 128, 8, 256], BF, isOutput=False)
    p_w2 = nc.declare_dram_parameter("W2", [NL, 8, 128, 32, 128], BF, isOutput=False)
    p_wh = nc.declare_dram_parameter("Wh", [128, 8, E], BF, isOutput=False)
    p_mask = nc.declare_dram_parameter("mask4", [2, 2, 128, 128], BF, isOutput=False)
    p_ones = nc.declare_dram_parameter("ones128", [128, 128], F32R, isOutput=False)
    p_out = nc.declare_dram_parameter("out", [TH, E], F32, isOutput=True)

    with tile.TileContext(nc) as tc:
        # ---------------- pools ----------------
        const = tc.alloc_tile_pool(name="const", bufs=1)
        persist = tc.alloc_tile_pool(name="persist", bufs=1)
        zpool = tc.alloc_tile_pool(name="zpool", bufs=1)
        big = tc.alloc_tile_pool(name="bigact", bufs=1)
        wpool = tc.alloc_tile_pool(name="wpool", bufs=6)
        w1pool = tc.alloc_tile_pool(name="w1pool", bufs=3)
        w8pool = tc.alloc_tile_pool(name="w8pool", bufs=3)
        whpool = tc.alloc_tile_pool(name="whpool", bufs=1)
        tmp = tc.alloc_tile_pool(name="tmp", bufs=3)
        stat = tc.alloc_tile_pool(name="stat", bufs=3)
        ptp = tc.alloc_tile_pool(name="ptp", bufs=7)
        dram = tc.alloc_tile_pool(name="dram", bufs=2, space="DRAM")
        pp_mm = tc.alloc_tile_pool(name="pp_mm", bufs=2, space="PSUM")
        pp_pair = tc.alloc_tile_pool(name="pp_pair", bufs=2, space="PSUM")
        pp_o = tc.alloc_tile_pool(name="pp_o", bufs=2, space="PSUM")

        ones128 = const.tile([128, 128], F32R)
        nc.sync.dma_start(ones128[:], p_ones.ap())
        onesb = const.tile([128, 128], BF)
        nc.vector.tensor_copy(onesb[:], ones128[:])
        eps_t = const.tile([128, 1], F32)
        nc.gpsimd.memset(eps_t[:], EPS)
        ones64 = const.tile([1, 64], F32)
        nc.gpsimd.memset(ones64[:], 1.0)
        # diagonal-block masks [k, src, dup(head), i]
        m4 = const.tile([128, 2, 2, 128], BF)
        nc.sync.dma_start(m4[:], p_mask.ap().rearrange("s u k i -> k s u i"))

        # peer half index for dynamic landing offsets
        pid = nc.scalar.partition_id()
        peer = (pid + 1) % 2
        offK = peer * (128 * KW)
        offV = peer * (128 * VW)

        # residual stream h^T, f32, D-chunk d at [:, d, :]
        h = persist.tile([128, DCH, TH], F32R)
        # Q^T zero-padded per head: head hd's 64 dims live in rows
        # (hd%2)*64.. of [:, hd, :]; the other 64 rows stay zero.
        QT2 = persist.tile([128, NH, TH], BF)
        nc.gpsimd.memset(QT2[:], 0.0)
        yT = persist.tile([128, DCH, TH], BF)    # attn out^T, rows=D
        # own-rank K^T / packed V (staged to the peer); peer-rank landing
        kloc = persist.tile([128, DCH, TH], BF)
        vloc = persist.tile([128, VW], BF)
        KTp = persist.tile([128, DCH, TH], BF)
        VA2p = persist.tile([128, VW], BF)
        # ones columns of the V packing (vloc is gathered; VA2p receives
        # the peer's copy with its ones already in place)
        nc.gpsimd.memset(vloc[:], 0.0)
        nc.gpsimd.memset(
            vloc[:, 0:NH * VHB].rearrange("p (x o) -> p x o", o=65)[:, :, 64:65],
            1.0)

        # ---------------- helpers ----------------
        def layernorm(z_out, src):
            """z_out (sbuf bf16 [128, DCH, TH]) = LayerNorm(src) in D-major."""
            s_b = pp_mm.tile([128, TH], F32, tag="mm")
            for d in range(DCH):
                nc.tensor.matmul(s_b[:], ones128[:], src[:, d, :],
                                 start=(d == 0), stop=(d == DCH - 1))
            q_b = pp_mm.tile([128, TH], F32, tag="mm")
            for d in range(DCH):
                sq = tmp.tile([128, TH], BF, tag="sq", bufs=2)
                nc.scalar.square(sq[:], src[:, d, :])
                nc.tensor.matmul(q_b[:], onesb[:], sq[:],
                                 start=(d == 0), stop=(d == DCH - 1))
            ss = stat.tile([128, TH], F32, tag="stat")
            nc.scalar.square(ss[:], s_b[:])
            u = stat.tile([128, TH], F32, tag="stat")
            nc.vector.scalar_tensor_tensor(u[:], ss[:], -1.0 / D, q_b[:],
                                           AluOp.mult, AluOp.add)
            rinv = stat.tile([128, TH], F32, tag="stat")
            nc.scalar.activation(rinv[:], u[:], Act.Sqrt, scale=1.0 / D,
                                 bias=eps_t[:])
            nc.vector.reciprocal_approx_fast(rinv[:], rinv[:])
            mr = u
            nc.vector.scalar_tensor_tensor(mr[:], s_b[:], 1.0 / D, rinv[:],
                                           AluOp.mult, AluOp.mult)
            for d in range(DCH):
                t = tmp.tile([128, TH], F32, tag="zt", bufs=2)
                nc.vector.tensor_tensor(t[:], src[:, d, :], rinv[:],
                                        AluOp.mult)
                nc.vector.tensor_tensor(z_out[:, d, :], t[:], mr[:],
                                        AluOp.subtract)

        # ---------------- embedding ----------------
        for r in range(4):
            wet = tmp.tile([128, 4, 128], BF, tag="tbf", bufs=2)
            nc.sync.dma_start(wet[:], p_we.ap()[r])
            ep = pp_mm.tile([128, TH], F32, tag="mm")
            for ec in range(4):
                lt = tmp.tile([128, TH], BF, tag="tbf", bufs=2)
                nc.sync.dma_start(lt[:], p_lcdT.ap()[ec * 128:(ec + 1) * 128, :])
                nc.tensor.matmul(ep[:], wet[:, ec, :], lt[:],
                                 start=(ec == 0), stop=(ec == 3))
            pt = tmp.tile([128, TH], F32, tag="t32", bufs=2)
            nc.sync.dma_start(pt[:], p_posT.ap()[r * 128:(r + 1) * 128, :])
            nc.vector.tensor_tensor(h[:, r, :], ep[:], pt[:], AluOp.add)
        actT = tmp.tile([AD, TH], F32, tag="t32", bufs=2)
        nc.sync.dma_start(actT[:], p_actT.ap())
        for r in range(4):
            wat = tmp.tile([AD, 128], F32, tag="t32", bufs=2)
            nc.sync.dma_start(wat[:], p_wa.ap()[:, r * 128:(r + 1) * 128])
            ap_ = pp_mm.tile([128, TH], F32, tag="mm")
            nc.tensor.matmul(ap_[:], wat[:], actT[:], start=True, stop=True)
            pt = tmp.tile([128, TH], F32, tag="t32", bufs=2)
            nc.sync.dma_start(pt[:], p_posT.ap()[(4 + r) * 128:(5 + r) * 128, :])
            nc.vector.tensor_tensor(h[:, 4 + r, :], ap_[:], pt[:], AluOp.add)

        # ---------------- transformer layers ----------------
        for l in range(NL):
            z1 = zpool.tile([128, DCH, TH], BF, tag="z", bufs=1)
            layernorm(z1, h)

            # ---- K^T own tokens -> kloc ----
            for r in range(DCH):
                wt = wpool.tile([128, DCH, 128], BF, tag="w")
                nc.sync.dma_start(wt[:], p_wk.ap()[l, r])
                kp = pp_mm.tile([128, TH], F32, tag="mm")
                for d in range(DCH):
                    nc.tensor.matmul(kp[:], wt[:, d, :], z1[:, d, :],
                                     start=(d == 0), stop=(d == DCH - 1))
                nc.vector.tensor_copy(kloc[:, r, :], kp[:])
            kin = dram.tile([128, KW], BF, tag="kin")
            nc.scalar.dma_start(kin[:], kloc[:])
            kout = dram.tile([2, 128, KW], BF, tag="kout")
            nc.gpsimd.collective_compute(
                "AllGather", AluOp.bypass,
                replica_groups=[[0, 1], [2, 3], [4, 5], [6, 7]],
                ins=[kin.opt()], outs=[kout.opt()])

            # ---- V own tokens, packed [V_h | ones] per head/block ----
            for nn in range(2):
                wvt = w8pool.tile([128, DCH, 512], BF, tag="w8")
                nc.sync.dma_start(wvt[:], p_wv.ap()[l, nn])
                for m in range(4):
                    vp = pp_mm.tile([128, 512], F32, tag="mm")
                    for d in range(DCH):
                        nc.tensor.matmul(
                            vp[:], z1[:, d, m * 128:(m + 1) * 128],
                            wvt[:, d, :],
                            start=(d == 0), stop=(d == DCH - 1))
                    nc.scalar.copy(
                        vloc[:, nn * 8 * VHB + m * 65:
                             nn * 8 * VHB + m * 65 + 8 * VHB].rearrange(
                            "p (hd x) -> p hd x", x=VHB)[:, :, 0:64],
                        vp.rearrange("p (hd dh) -> p hd dh", dh=DH))
            vin = dram.tile([128, VW], BF, tag="vin")
            nc.scalar.dma_start(vin[:], vloc[:])
            vout = dram.tile([2, 128, VW], BF, tag="vout")
            nc.gpsimd.collective_compute(
                "AllGather", AluOp.bypass,
                replica_groups=[[0, 1], [2, 3], [4, 5], [6, 7]],
                ins=[vin.opt()], outs=[vout.opt()])

            # ---- Q^T own tokens into zero-padded per-head slots ----
            for r in range(DCH):
                wt = wpool.tile([128, DCH, 128], BF, tag="w")
                nc.sync.dma_start(wt[:], p_wq.ap()[l, r])
                qp = pp_mm.tile([128, TH], F32, tag="mm")
                for d in range(DCH):
                    nc.tensor.matmul(qp[:], wt[:, d, :], z1[:, d, :],
                                     start=(d == 0), stop=(d == DCH - 1))
                nc.vector.tensor_copy(QT2[0:64, 2 * r, :], qp[0:64, :])
                nc.vector.tensor_copy(QT2[64:128, 2 * r + 1, :], qp[64:128, :])

            # ---- land the PEER half of the gathers (dynamic offset),
            # split and interleaved so head 0 unblocks after piece 1 ----
            kb = kout[0].rearrange("p (d t) -> p d t", t=TH)
            vb = vout[0]
            vmid = 8 * VHB
            for piece in range(2):
                kslice = kb[:, 4 * piece:4 * piece + 4]
                nc.scalar.dma_start(
                    KTp[:, 4 * piece:4 * piece + 4],
                    dataclasses.replace(kslice, offset=kslice.offset + offK,
                                        dep_tracking_offset=kslice.offset))
                vlo, vhi = (0, vmid) if piece == 0 else (vmid, VW)
                vslice = vb[:, vlo:vhi]
                nc.scalar.dma_start(
                    VA2p[:, vlo:vhi],
                    dataclasses.replace(vslice, offset=vslice.offset + offV,
                                        dep_tracking_offset=vslice.offset))

            # ---- attention ----
            for hd in range(NH):
                rc, ro = hd // 2, (hd % 2) * 64
                o_p = pp_o.tile([128, TH], F32, tag="o")
                p2s = []
                for m in range(4):
                    W = TH - m * 128
                    s2 = pp_pair.tile([128, 2, TH], F32, tag="pair")
                    for si, srcK in ((0, kloc), (1, KTp)):
                        nc.tensor.matmul(
                            s2[:, si, 0:W],
                            srcK[:, rc, m * 128:(m + 1) * 128],
                            QT2[:, hd, m * 128:TH],
                            start=True, stop=True)
                    p2 = ptp.tile([128, 2, TH], BF, tag="p2")
                    nc.scalar.activation(p2[:, :, 0:W], s2[:, :, 0:W],
                                         Act.Exp,
                                         scale=1.0 / float(np.sqrt(DH)))
                    nc.vector.tensor_tensor(p2[:, :, 0:128],
                                            p2[:, :, 0:128],
                                            m4[:, :, 0, :], AluOp.mult)
                    p2s.append(p2)
                for m in range(4):
                    W = TH - m * 128
                    for si, srcV in ((0, vloc), (1, VA2p)):
                        nc.tensor.matmul(
                            o_p[:, m * 128:TH],
                            srcV[:, hd * VHB + m * 65:
                                 hd * VHB + m * 65 + 128],
                            p2s[m][:, si, 0:W],
                            start=(m == 0 and si == 0),
                            stop=(m == 3 and si == 1))
                inv1 = stat.tile([1, TH], F32, tag="den1", bufs=1)
                nc.vector.tensor_copy(inv1[:], o_p[64:65, :])
                nc.vector.reciprocal_approx_fast(inv1[:], inv1[:])
                invO = stat.tile([64, TH], F32, tag="den", bufs=1)
                nc.gpsimd.partition_broadcast(invO[:], inv1[0:1, :])
                nc.vector.tensor_tensor(yT[ro:ro + 64, rc, :], o_p[0:64, :],
                                        invO[:], AluOp.mult)

            # ---- proj + residual ----
            for r in range(DCH):
                wt = wpool.tile([128, DCH, 128], BF, tag="w")
                nc.sync.dma_start(wt[:], p_wp.ap()[l, r])
                pp = pp_mm.tile([128, TH], F32, tag="mm")
                for d in range(DCH):
                    nc.tensor.matmul(pp[:], wt[:, d, :], yT[:, d, :],
                                     start=(d == 0), stop=(d == DCH - 1))
                nc.vector.tensor_tensor(h[:, r, :], h[:, r, :], pp[:],
                                        AluOp.add)

            # ---- MLP ----
            z2 = zpool.tile([128, DCH, TH], BF, tag="z", bufs=1)
            layernorm(z2, h)
            aT = big.tile([128, 32, TH], BF, tag="aT")
            w2ts = []
            for r in range(2):
                w2t = w8pool.tile([128, FF // 128, 128], BF, tag="w8")
                nc.scalar.dma_start(w2t[:], p_w2.ap()[l, r])
                w2ts.append(w2t)
            for fi in range(16):
                w1t = w1pool.tile([128, DCH, 256], BF, tag="w1p")
                nc.sync.dma_start(w1t[:], p_w1.ap()[l, fi])
                fp = pp_pair.tile([128, 2, TH], F32, tag="pair")
                for j in range(2):
                    for d in range(DCH):
                        nc.tensor.matmul(
                            fp[:, j, :], w1t[:, d, j * 128:(j + 1) * 128],
                            z2[:, d, :],
                            start=(d == 0), stop=(d == DCH - 1))
                nc.scalar.activation(aT[:, 2 * fi:2 * fi + 2, :], fp[:],
                                     Act.Gelu)
            for r in range(DCH):
                if r + 2 < DCH:
                    w2t = w8pool.tile([128, FF // 128, 128], BF, tag="w8")
                    nc.scalar.dma_start(w2t[:], p_w2.ap()[l, r + 2])
                    w2ts.append(w2t)
                w2t = w2ts[r]
                mp = pp_mm.tile([128, TH], F32, tag="mm")
                for fc in range(32):
                    nc.tensor.matmul(mp[:], w2t[:, fc, :], aT[:, fc, :],
                                     start=(fc == 0), stop=(fc == 31))
                nc.vector.tensor_tensor(h[:, r, :], h[:, r, :], mp[:],
                                        AluOp.add)

        # ---------------- final LN + head ----------------
        zf = zpool.tile([128, DCH, TH], BF, tag="z", bufs=1)
        layernorm(zf, h)
        wht = whpool.tile([128, DCH, E], BF, tag="wh")
        nc.sync.dma_start(wht[:], p_wh.ap())
        for tb in range(4):
            op_ = pp_mm.tile([128, E], F32, tag="mm")
            for d in range(DCH):
                nc.tensor.matmul(
                    op_[:],
                    zf[:, d, tb * 128:(tb + 1) * 128],
                    wht[:, d, :],
                    start=(d == 0), stop=(d == DCH - 1))
            ot = tmp.tile([128, E], F32, tag="t32", bufs=2)
            nc.scalar.copy(ot[:], op_[:])
            nc.sync.dma_start(p_out.ap()[tb * 128:(tb + 1) * 128, :], ot[:])

        for _pool in reversed((const, persist, zpool, big, wpool, w1pool,
                               w8pool, whpool, tmp, stat, ptp, dram, pp_mm,
                               pp_pair, pp_o)):
            _pool.release()

    nc.compile()
    return nc


def _get_program():
    if "nc" not in _cache:
        _cache["nc"] = _build_program()
    return _cache["nc"]


def _bf16(x):
    return np.ascontiguousarray(np.asarray(x).astype(ml_dtypes.bfloat16))


def _f32(x):
    return np.ascontiguousarray(np.asarray(x).astype(np.float32))


def _pack_sq(w):
    """[Din,Dout] -> [r, p, dc, c] = w[dc*128+p, r*128+c] per-r contiguous."""
    din, dout = w.shape
    return w.reshape(din // 128, 128, dout // 128, 128).transpose(2, 1, 0, 3)


def _pack_w(w, cols):
    """[Din,Dout] -> [i, p, dc, cols] = w[dc*128+p, i*cols+c]."""
    din, dout = w.shape
    return w.reshape(din // 128, 128, dout // cols, cols).transpose(2, 1, 0, 3)


def make_in_maps(inputs):
    lcd = np.asarray(inputs["lcd"], np.float32).reshape(B, T, E)
    lcd_shift = np.concatenate(
        [np.zeros((B, 1, E), np.float32), lcd[:, :-1]], axis=1)
    action = np.asarray(inputs["action"], np.float32)
    pos = np.asarray(inputs["pos_emb"], np.float32)[0]          # [T, D]

    shared = {
        "W_embed": _bf16(_pack_w(np.asarray(inputs["W_embed"]), 128)),
        "W_act": _f32(inputs["W_act"]),
        "Wq": _bf16(np.stack([_pack_sq(w) for w in np.asarray(inputs["Wq"])])),
        "Wk": _bf16(np.stack([_pack_sq(w) for w in np.asarray(inputs["Wk"])])),
        "Wv": _bf16(np.stack([_pack_w(w, 512) for w in np.asarray(inputs["Wv"])])),
        "Wp": _bf16(np.stack([_pack_sq(w) for w in np.asarray(inputs["Wp"])])),
        "W1": _bf16(np.stack([_pack_w(w, 256) for w in np.asarray(inputs["W1"])])),
        "W2": _bf16(np.stack([_pack_sq(w) for w in np.asarray(inputs["W2"])])),
        "Wh": _bf16(np.asarray(inputs["Wh"]).reshape(8, 128, E).transpose(1, 0, 2)),
        "ones128": np.ones((128, 128), np.float32),
    }

    kk, ii = np.arange(128)[:, None], np.arange(128)[None, :]
    in_maps = []
    for c in range(NC):
        b, r = c // 2, c % 2
        tok = np.arange(T)[r::2]                                # own tokens
        # diagonal-block masks [src(own/peer), dup, k, i]:
        # own: k <= i; peer: 2k + (1-r) <= 2i + r
        mown = (kk <= ii).astype(np.float32)
        mpeer = (2 * kk + (1 - r) <= 2 * ii + r).astype(np.float32)
        mask4 = np.stack([np.stack([mown, mown]), np.stack([mpeer, mpeer])])
        in_maps.append(dict(
            shared,
            lcdT=_bf16(lcd_shift[b, tok].T),                    # [E, TH]
            actT=_f32(action[b, tok].T),                        # [AD, TH]
            posT=_f32(pos[tok].T),                              # [D, TH]
            mask4=_bf16(mask4),
        ))
    return in_maps


def assemble(results):
    out = np.empty((B, T, E), np.float32)
    for c in range(NC):
        b, r = c // 2, c % 2
        out[b, r::2] = results[c]["out"]
    return out


def kernel(**inputs):
    nc = _get_program()
    in_maps = make_in_maps(inputs)
    res = run_bass_kernel_spmd(nc, in_maps, list(range(NC)))
    return assemble(res.results)
